# revision 22
# baseline (speedup 1.0000x reference)
"""Causal MQA kernel for Trainium2, SPMD over 8 NeuronCores.

Sharding: core i = (batch b = i//4, head-group hg = i%4). Each core computes
K/V projections for its batch locally (no collectives), the q projection for
its 4 heads, and causal attention for those heads; it writes the [T, 512]
fp16 output slice y[b, :, hg*512:(hg+1)*512]. The host concatenates slices.

Device algorithm (per core, T processed in 4 chunks of QC=512 queries):
  - Projections run in fp8e4 with DoubleRow perf mode (2 contraction k-tiles
    per instruction, 2x PE throughput): weights are scaled by S=32 on the
    host to avoid fp8 subnormals; the S^2 factor folds into the exp scale
    and the S factor on V folds into the softmax-denominator constant.
  - Scores S^T[k, q] = matmul(lhsT=kT16 tile, rhs=qT16 chunk) in fp16.
  - P^T = exp(S^T * scale) emitted directly to fp8 (ACT), one instruction
    per key-tile pair; causal masking on diagonal tiles via gpsimd
    affine_select on the [128,128] triangle + a gap memset (both on Pool).
  - PV: y^T accumulates in PSUM via fp8 DoubleRow over key-tile pairs
    (lhsT = v8 pair, rhs = pt8 pair).
  - Softmax denominators: fp8 DoubleRow ones-matmul (lhsT = const 0.5) into
    a [32, 512] PSUM accumulator per head, accumulated across pairs.
  - fp8 error control: chunk 0 (queries 0-511, keys 0-511) runs an
    accurate path -- three-term q/k/v projections (~0.5% error) with fp16
    scores/pt16/PV -- because its low-key-count rows get no softmax
    averaging; later rows tolerate the fp8 noise (measured ~1.5e-2 max).
  - All transposes (v, denominators, y^T) go through the DMA XBAR
    (dma_start_transpose), keeping the PE free; the tail is deferred one
    chunk and does reciprocal + per-partition scale-multiply on DVE.
"""

import math
from contextlib import ExitStack

import numpy as np
import ml_dtypes

import concourse.bass as bass
import concourse.mybir as mybir
import concourse.tile as tile
from concourse import bacc
from concourse.bass_utils import run_bass_kernel_spmd

F32 = mybir.dt.float32
F16 = mybir.dt.float16
F8 = mybir.dt.float8e4
E4M3 = ml_dtypes.float8_e4m3

P = 128  # partitions
HD = 128  # head dim
QC = 512  # query-chunk width (one fp32 PSUM bank)
N_CORES = 8
HPC = 4  # query heads per core
NB = 4  # head groups (cores per batch)
S = 32.0  # host-side weight scale (fp8 subnormal avoidance)
ALPHA = 1.0 / 64.0  # ysum copy scale (fp16 overflow avoidance)
BETA = S * ALPHA  # denominator matmul constant = 0.5 (exact in fp8)
DR = mybir.MatmulPerfMode.DoubleRow

PHASE_MARKS = []


def _mark(nc, name):
    n = int(nc.get_next_instruction_name().split("-")[-1])
    PHASE_MARKS.append((n, name))


def build_nc(T, C):
    NQC = T // QC  # query chunks (4)
    NCC = C // P  # contraction chunks (16)
    KTQ = QC // P  # key tiles per query chunk (4)
    NKT = T // P  # key tiles total (16)
    exp_scale = 1.0 / (math.sqrt(HD) * S * S)

    nc = bacc.Bacc("TRN2", target_bir_lowering=False, debug=False,
                   num_devices=N_CORES)
    x8a = nc.dram_tensor("x8a", [C, T], F8, kind="ExternalInput").ap()
    x8b0 = nc.dram_tensor("x8b0", [C, QC], F8, kind="ExternalInput").ap()
    # pre-shuffled on host to [P, ...] so weight DMAs are contiguous copies
    NCC_ = C // P
    wq8 = nc.dram_tensor("wq8", [P, NCC_ * 2 * HPC * HD], F8,
                         kind="ExternalInput").ap()
    # wkv8 = [wk8a | wk8b | wv8a | wv8b], each [P, NCC*HD]
    wkv8 = nc.dram_tensor("wkv8", [P, 4 * NCC_ * HD], F8,
                          kind="ExternalInput").ap()
    y = nc.dram_tensor("y", [T, HPC * HD], F16, kind="ExternalOutput").ap()

    with tile.TileContext(nc) as tc, ExitStack() as ctx, \
            nc.allow_low_precision(reason="fp8 operands feed the PE; accumulation stays fp32 in PSUM"):
        consts = ctx.enter_context(tc.tile_pool(name="consts", bufs=1))
        ones8 = consts.tile([P, 2, 32], F8, tag="ones8")
        nc.gpsimd.memset(ones8, BETA)
        ones16 = consts.tile([P, 32], F16, tag="ones16")
        nc.gpsimd.memset(ones16, BETA)

        # ---- persistent SBUF ----
        big = ctx.enter_context(tc.tile_pool(name="big", bufs=1))
        x8a_sb = big.tile([P, NQC, NCC, QC], F8, tag="x8a")
        x8b0_sb = big.tile([P, NCC, QC], F8, tag="x8b0")
        wq8_sb = big.tile([P, NCC, 2 * HPC * HD], F8, tag="wq8")
        wkv8_sb = big.tile([P, 4, NCC, HD], F8, tag="wkv8")
        kT16 = big.tile([P, T], F16, tag="kT16")
        kT016 = big.tile([P, QC], F16, tag="kT016")
        v8 = big.tile([P, NKT, HD], F8, tag="v8")
        v016 = big.tile([P, KTQ, HD], F16, tag="v016")

        # ---- pools ----
        qT_pool = ctx.enter_context(tc.tile_pool(name="qT", bufs=2))
        pt_pool = ctx.enter_context(tc.tile_pool(name="pt", bufs=6))
        pt16_pool = ctx.enter_context(tc.tile_pool(name="pt16", bufs=3))
        vt_pool = ctx.enter_context(tc.tile_pool(name="vt", bufs=2))
        vtt_pool = ctx.enter_context(tc.tile_pool(name="vtt", bufs=2))
        ysum_pool = ctx.enter_context(tc.tile_pool(name="ysum", bufs=10))
        ysumt_pool = ctx.enter_context(tc.tile_pool(name="ysumt", bufs=5))
        sums_sb_pool = ctx.enter_context(tc.tile_pool(name="ssb", bufs=3))
        sumst_pool = ctx.enter_context(tc.tile_pool(name="sumst", bufs=3))
        yout_pool = ctx.enter_context(tc.tile_pool(name="yout", bufs=2))
        recip_pool = ctx.enter_context(tc.tile_pool(name="recip", bufs=3))

        # PSUM (8 banks): st 2x[128,2,512] = 4, y 2x[128,512] = 2,
        # sums 2x[32,512] = 2. Projections share st slots.
        st_pp = ctx.enter_context(tc.tile_pool(name="st_pp", bufs=2,
                                               space="PSUM"))
        y_pp = ctx.enter_context(tc.tile_pool(name="y_pp", bufs=2,
                                              space="PSUM"))
        sums_pp = ctx.enter_context(tc.tile_pool(name="sums_pp", bufs=2,
                                                 space="PSUM"))

        # ---- input DMAs: one queue, priority order, so the startup-critical
        # transfers (wkv8 + x chunk 0) are not bandwidth-starved ----
        xr = x8a.rearrange("(cc p) t -> p cc t", p=P)
        wkv_r = wkv8.rearrange("p (f cc d) -> p f cc d", f=4, cc=NCC)
        nc.sync.dma_start(out=wkv8_sb[:, 0], in_=wkv_r[:, 0])  # wk8a first
        nc.sync.dma_start(out=x8a_sb[:, 0, 0:NCC // 2], in_=xr[:, 0:NCC // 2, 0:QC])
        nc.sync.dma_start(out=x8a_sb[:, 0, NCC // 2:], in_=xr[:, NCC // 2:, 0:QC])
        nc.sync.dma_start(out=wkv8_sb[:, 1:], in_=wkv_r[:, 1:])
        nc.sync.dma_start(out=x8b0_sb,
                          in_=x8b0.rearrange("(cc p) t -> p cc t", p=P))
        nc.sync.dma_start(out=wq8_sb,
                          in_=wq8.rearrange("p (cc d) -> p cc d", cc=NCC))
        for tq in range(1, NQC):
            nc.sync.dma_start(out=x8a_sb[:, tq],
                              in_=xr[:, :, tq * QC:(tq + 1) * QC])

        def dr_proj(ps, w_sl, x_sl, first, last):
            # 8 DoubleRow matmuls: contraction C in pairs of 128-row tiles
            for c4 in range(NCC // 2):
                nc.tensor.matmul(
                    ps, lhsT=w_sl(c4), rhs=x_sl(c4),
                    start=(first and c4 == 0), stop=(last and c4 == NCC // 2 - 1),
                    perf_mode=DR)

        def wkv_slice(f):
            return lambda c4: wkv8_sb[:, f, 2 * c4:2 * c4 + 2]

        def wq_slice(term, h):
            off = term * HPC * HD + h * HD
            return lambda c4: wq8_sb[:, 2 * c4:2 * c4 + 2, off:off + HD]

        def x_slice(tq):
            return lambda c4: x8a_sb[:, tq, 2 * c4:2 * c4 + 2]

        def x0b_slice():
            return lambda c4: x8b0_sb[:, 2 * c4:2 * c4 + 2]

        def k_chunk(tq):
            ps = st_pp.tile([P, QC], F32, tag="st")
            dr_proj(ps, wkv_slice(0), x_slice(tq), True, True)
            nc.vector.tensor_copy(kT16[:, tq * QC:(tq + 1) * QC], ps)

        def v_chunk(tq, two_term):
            ps = st_pp.tile([P, QC], F32, tag="st")
            dr_proj(ps, wkv_slice(2), x_slice(tq), True, not two_term)
            if two_term:
                dr_proj(ps, wkv_slice(3), x_slice(tq), False, False)
                dr_proj(ps, wkv_slice(2), x0b_slice(), False, True)
            vt16 = vt_pool.tile([P, QC], F16, tag="vt")
            nc.vector.tensor_copy(vt16, ps)
            if two_term:
                nc.sync.dma_start_transpose(v016, vt16)
            else:
                vtt = vtt_pool.tile([P, KTQ, HD], F16, tag="vtt")
                nc.sync.dma_start_transpose(vtt, vt16)
                nc.vector.tensor_copy(v8[:, tq * KTQ:(tq + 1) * KTQ], vtt)

        with nc.named_scope("kv0"):
            _mark(nc, "kv0")
            ps0 = st_pp.tile([P, QC], F32, tag="st")
            dr_proj(ps0, wkv_slice(0), x_slice(0), True, False)
            dr_proj(ps0, wkv_slice(1), x_slice(0), False, False)
            dr_proj(ps0, wkv_slice(0), x0b_slice(), False, True)
            nc.vector.tensor_copy(kT016, ps0)
            v_chunk(0, True)

        # ---- Q projection per (chunk, head) ----
        qts = {}

        def emit_qproj(tq):
            _mark(nc, f"qproj{tq}")
            with nc.named_scope(f"qproj{tq}"):
                qTq = qT_pool.tile([P, HPC, QC], F16, tag="qT")
                for h in range(HPC):
                    ps = st_pp.tile([P, QC], F32, tag="st")
                    dr_proj(ps, wq_slice(0, h), x_slice(tq), True, tq != 0)
                    if tq == 0:  # three-term q for the low-key-count chunk
                        dr_proj(ps, wq_slice(1, h), x_slice(0), False, False)
                        dr_proj(ps, wq_slice(0, h), x0b_slice(), False, True)
                    nc.vector.tensor_copy(qTq[:, h], ps)
                qts[tq] = qTq

        pending_tails = []
        chunk_tail_state = {}

        def emit_tail(tq, hp, ysum2, sums2):
            # Per head-pair, deferred one chunk. PE-free: transposes via the
            # DMA XBAR. One sums transpose + one output DMA per chunk.
            with nc.named_scope(f"ltail{tq}p{hp}"):
                _mark(nc, f"q{tq}:ltail{hp}")
                if tq not in chunk_tail_state:
                    yo = yout_pool.tile([P, KTQ, HPC, HD], F16, tag="yo")
                    chunk_tail_state[tq] = yo
                yo = chunk_tail_state[tq]
                sumst = sumst_pool.tile([P, KTQ, 64], F16, tag="sumst")
                nc.sync.dma_start_transpose(
                    sumst, sums2[hp * 64:hp * 64 + 64, :])
                rt = recip_pool.tile([P, KTQ, 2], F32, tag="recip")
                for i in range(2):
                    nc.vector.reciprocal(rt[:, :, i:i + 1],
                                         sumst[:, :, i * 32:i * 32 + 1])
                yst = ysumt_pool.tile([P, 2 * KTQ, HD], F16, tag="yst")
                nc.scalar.dma_start_transpose(
                    yst, ysum2.rearrange("p two q -> p (two q)"))
                for i in range(2):
                    for qt in range(KTQ):
                        nc.vector.tensor_scalar_mul(
                            yo[:, qt, 2 * hp + i], yst[:, i * KTQ + qt],
                            rt[:, qt, i:i + 1])
                if hp == 1:
                    ydst = y.rearrange("(nq qt p) (hh d) -> nq p qt hh d",
                                       qt=KTQ, p=P, hh=HPC)[tq]
                    nc.sync.dma_start(out=ydst, in_=yo)

        def emit_attn(tq):
            # ---- causal attention for this query chunk ----
            # Head PAIRS: the PE alternates score and PV matmuls between the
            # two heads so each head's exp chain hides under the other's
            # matmuls. Chunk 0 uses the accurate fp16 path.
            last_chunk = tq == NQC - 1
            fp16_path = tq == 0
            nkt = (tq + 1) * KTQ
            ngr = nkt // 2  # key-tile pairs
            qTq = qts.pop(tq)
            sums2 = sums_sb_pool.tile([P, QC], F16, tag="ssb")
            kt_src = kT016 if fp16_path else kT16

            def s_mm(h, g):
                # scores + exp (+ diag mask) for key-tile pair g
                st = st_pp.tile([P, 2, QC], F32, tag="st")
                if fp16_path:
                    pt = pt16_pool.tile([P, 2, QC], F16, tag="pt16")
                else:
                    pt = pt_pool.tile([P, 2, QC], F8, tag="pt")
                qrhs = qTq[:, h]
                offs = []
                for u in range(2):
                    kt = 2 * g + u
                    off = max((kt - KTQ * tq) * P, 0)
                    offs.append(off)
                    nc.tensor.matmul(
                        st[:, u, off:],
                        lhsT=kt_src[:, kt * P:(kt + 1) * P],
                        rhs=qrhs[:, off:], start=True, stop=True)
                o0, o1 = offs
                nc.scalar.activation(pt[:, :, o0:], st[:, :, o0:],
                                     mybir.ActivationFunctionType.Exp,
                                     scale=exp_scale)
                if 2 * g + 1 >= KTQ * tq:  # pair contains diagonal tiles
                    if o1 > o0:
                        nc.gpsimd.memset(pt[:, 1, o0:o1], 0.0)
                    for u, off in enumerate(offs):
                        nc.gpsimd.affine_select(
                            out=pt[:, u, off:off + P],
                            in_=pt[:, u, off:off + P],
                            pattern=[[1, P]],
                            compare_op=mybir.AluOpType.is_ge,
                            fill=0.0, base=0, channel_multiplier=-1)
                return pt, o0

            for hp in range(HPC // 2):
              pair = (2 * hp, 2 * hp + 1)
              with nc.named_scope(f"attn{tq}p{hp}"):
                y_psh = {}
                s_psh = {}
                pts = {}
                for h in pair:
                    y_ps = y_pp.tile([P, QC], F32, tag="y")
                    y_psh[h] = y_ps
                    s_ps = sums_pp.tile([32, QC], F32, tag="sums")
                    s_psh[h] = s_ps
                    pts[h] = {0: s_mm(h, 0)}
                for g in range(ngr):
                    _mark(nc, f"q{tq}:att{hp}g{g}")
                    if g + 1 < ngr:
                        for h in pair:
                            pts[h][g + 1] = s_mm(h, g + 1)
                    for h in pair:
                        pt, o0 = pts[h].pop(g)
                        first, last = g == 0, g == ngr - 1
                        if fp16_path:
                            for u in range(2):
                                kt = 2 * g + u
                                off = max((kt - KTQ * tq) * P, 0)
                                nc.tensor.matmul(
                                    y_psh[h][:, off:], lhsT=v016[:, kt],
                                    rhs=pt[:, u, off:],
                                    start=(kt == 0), stop=(kt == nkt - 1),
                                    skip_group_check=True)
                                nc.tensor.matmul(
                                    s_psh[h][:, off:], lhsT=ones16,
                                    rhs=pt[:, u, off:],
                                    start=(kt == 0), stop=(kt == nkt - 1),
                                    skip_group_check=True)
                        else:
                            v_sl = v8.rearrange("p (gg two) d -> p gg two d",
                                                two=2)[:, g]
                            nc.tensor.matmul(
                                y_psh[h][:, o0:], lhsT=v_sl,
                                rhs=pt[:, :, o0:],
                                start=first, stop=last, perf_mode=DR,
                                skip_group_check=True)
                            nc.tensor.matmul(
                                s_psh[h][:, o0:], lhsT=ones8,
                                rhs=pt[:, :, o0:],
                                start=first, stop=last, perf_mode=DR,
                                skip_group_check=True)
                _mark(nc, f"q{tq}:tail{hp}")
                ysum2 = ysum_pool.tile([P, 2, QC], F16, tag="ysum")
                for i, h in enumerate(pair):
                    nc.vector.tensor_scalar_mul(ysum2[:, i], y_psh[h], ALPHA)
                    nc.vector.tensor_copy(sums2[h * 32:h * 32 + 1, :],
                                          s_psh[h][0:1, :])
                pending_tails.append((tq, hp, ysum2, sums2))
                # keep <= 2 pending (pipelines tails under the next chunk's
                # matmuls); on the last chunk drain eagerly so each tail
                # overlaps the remaining pairs' matmuls
                limit = 0 if last_chunk else 2
                while len(pending_tails) > limit:
                    emit_tail(*pending_tails.pop(0))

        # ---- emission order: minimal work before attn0 (K0s/V0s are only
        # needed from attn1 on, so they ride with kv1) ----
        emit_qproj(0)
        emit_attn(0)
        for tq in range(1, NQC):
            with nc.named_scope(f"kv{tq}"):
                _mark(nc, f"kv{tq}")
                if tq == 1:
                    k_chunk(0)
                    v_chunk(0, False)
                k_chunk(tq)
                v_chunk(tq, False)
            emit_qproj(tq)
            emit_attn(tq)

    nc.compile()
    return nc


_cache = {}


def _get_nc(T, C):
    key = (T, C)
    if key not in _cache:
        _cache[key] = build_nc(T, C)
    return _cache[key]


def prepare_in_maps(x, w_kv, w_q):
    x = np.asarray(x, dtype=np.float32)
    w_kv = np.asarray(w_kv, dtype=np.float32)
    w_q = np.asarray(w_q, dtype=np.float32)
    B, T, C = x.shape

    NCC = C // P

    def two_term(w):  # [C, D] scaled two-term fp8
        ws = np.ascontiguousarray(w) * S
        a = ws.astype(E4M3)
        b = (ws - a.astype(np.float32)).astype(E4M3)
        return a, b

    def shuffle(w8):  # [C, D] -> [P, NCC*D] matching [p, cc, d] tiles
        D = w8.shape[1]
        return w8.reshape(NCC, P, D).transpose(1, 0, 2).reshape(P, NCC * D)

    wk = w_kv[:HD].T  # [C, HD]
    wv = w_kv[HD:].T
    wk8a, wk8b = two_term(wk)
    wv8a, wv8b = two_term(wv)
    wkv8 = np.ascontiguousarray(np.concatenate(
        [shuffle(w) for w in (wk8a, wk8b, wv8a, wv8b)], axis=1))

    in_maps = []
    for i in range(N_CORES):
        b, hg = divmod(i, NB)
        xT = np.ascontiguousarray(x[b].T)  # [C, T]
        x8a = xT.astype(E4M3)
        x8b0 = (xT[:, :QC] - x8a[:, :QC].astype(np.float32)).astype(E4M3)
        wqs = w_q[hg * HPC * HD:(hg + 1) * HPC * HD].T  # [C, 512]
        wq8a, wq8b = two_term(wqs)
        # interleave per-cc: [P, NCC, 2*512] flattened
        sa = shuffle(wq8a).reshape(P, NCC, HPC * HD)
        sb = shuffle(wq8b).reshape(P, NCC, HPC * HD)
        wq8 = np.ascontiguousarray(
            np.concatenate([sa, sb], axis=2).reshape(P, NCC * 2 * HPC * HD))
        in_maps.append({"x8a": x8a, "x8b0": np.ascontiguousarray(x8b0),
                        "wq8": wq8, "wkv8": wkv8})
    return in_maps


def gather_output(results, B, T, C):
    out = np.empty((B, T, C), np.float32)
    for i in range(N_CORES):
        b, hg = divmod(i, NB)
        out[b, :, hg * HPC * HD:(hg + 1) * HPC * HD] = \
            results[i]["y"].astype(np.float32)
    return out


def kernel(x, w_kv, w_q):
    x = np.asarray(x)
    B, T, C = x.shape
    nc = _get_nc(T, C)
    in_maps = prepare_in_maps(x, w_kv, w_q)
    res = run_bass_kernel_spmd(nc, in_maps, list(range(N_CORES)))
    return gather_output(res.results, B, T, C)


# revision 25
# speedup vs baseline: 1.0033x; 1.0033x over previous
"""Causal MQA kernel for Trainium2, SPMD over 8 NeuronCores.

Sharding: core i = (batch b = i//4, head-group hg = i%4). Each core computes
K/V projections for its batch locally (no collectives), the q projection for
its 4 heads, and causal attention for those heads; it writes the [T, 512]
fp16 output slice y[b, :, hg*512:(hg+1)*512]. The host concatenates slices.

Device algorithm (per core, T processed in 4 chunks of QC=512 queries):
  - Projections run in fp8e4 with DoubleRow perf mode (2 contraction k-tiles
    per instruction, 2x PE throughput): weights are scaled by S=32 on the
    host to avoid fp8 subnormals; the S^2 factor folds into the exp scale
    and the S factor on V folds into the softmax-denominator constant.
  - Scores S^T[k, q] = matmul(lhsT=kT16 tile, rhs=qT16 chunk) in fp16.
  - P^T = exp(S^T * scale) emitted directly to fp8 (ACT), one instruction
    per key-tile pair; causal masking on diagonal tiles via gpsimd
    affine_select on the [128,128] triangle + a gap memset (both on Pool).
  - PV: y^T accumulates in PSUM via fp8 DoubleRow over key-tile pairs
    (lhsT = v8 pair, rhs = pt8 pair).
  - Softmax denominators: fp8 DoubleRow ones-matmul (lhsT = const 0.5) into
    a [32, 512] PSUM accumulator per head, accumulated across pairs.
  - fp8 error control: chunk 0 (queries 0-511, keys 0-511) runs an
    accurate path -- three-term q/k/v projections (~0.5% error) with fp16
    scores/pt16/PV -- because its low-key-count rows get no softmax
    averaging; later rows tolerate the fp8 noise (measured ~1.5e-2 max).
  - All transposes (v, denominators, y^T) go through the DMA XBAR
    (dma_start_transpose), keeping the PE free; the tail is deferred one
    chunk and does reciprocal + per-partition scale-multiply on DVE.
"""

import math
from contextlib import ExitStack

import numpy as np
import ml_dtypes

import concourse.bass as bass
import concourse.mybir as mybir
import concourse.tile as tile
from concourse import bacc
from concourse.bass_utils import run_bass_kernel_spmd

F32 = mybir.dt.float32
F16 = mybir.dt.float16
F8 = mybir.dt.float8e4
E4M3 = ml_dtypes.float8_e4m3

P = 128  # partitions
HD = 128  # head dim
QC = 512  # query-chunk width (one fp32 PSUM bank)
N_CORES = 8
HPC = 4  # query heads per core
NB = 4  # head groups (cores per batch)
S = 32.0  # host-side weight scale (fp8 subnormal avoidance)
ALPHA = 1.0 / 64.0  # ysum copy scale (fp16 overflow avoidance)
BETA = S * ALPHA  # denominator matmul constant = 0.5 (exact in fp8)
DR = mybir.MatmulPerfMode.DoubleRow

PHASE_MARKS = []


def _mark(nc, name):
    n = int(nc.get_next_instruction_name().split("-")[-1])
    PHASE_MARKS.append((n, name))


def build_nc(T, C):
    NQC = T // QC  # query chunks (4)
    NCC = C // P  # contraction chunks (16)
    KTQ = QC // P  # key tiles per query chunk (4)
    NKT = T // P  # key tiles total (16)
    exp_scale = 1.0 / (math.sqrt(HD) * S * S)

    nc = bacc.Bacc("TRN2", target_bir_lowering=False, debug=False,
                   num_devices=N_CORES)
    x8a = nc.dram_tensor("x8a", [C, T], F8, kind="ExternalInput").ap()
    x8b0 = nc.dram_tensor("x8b0", [C, QC], F8, kind="ExternalInput").ap()
    # pre-shuffled on host to [P, ...] so weight DMAs are contiguous copies
    NCC_ = C // P
    wq8 = nc.dram_tensor("wq8", [P, NCC_ * 2 * HPC * HD], F8,
                         kind="ExternalInput").ap()
    # wkv8 = [wk8a | wk8b | wv8a | wv8b], each [P, NCC*HD]
    wkv8 = nc.dram_tensor("wkv8", [P, 4 * NCC_ * HD], F8,
                          kind="ExternalInput").ap()
    y = nc.dram_tensor("y", [T, HPC * HD], F16, kind="ExternalOutput").ap()

    with tile.TileContext(nc) as tc, ExitStack() as ctx, \
            nc.allow_low_precision(reason="fp8 operands feed the PE; accumulation stays fp32 in PSUM"):
        consts = ctx.enter_context(tc.tile_pool(name="consts", bufs=1))
        ones8 = consts.tile([P, 2, 32], F8, tag="ones8")
        nc.gpsimd.memset(ones8, BETA)
        ones16 = consts.tile([P, 32], F16, tag="ones16")
        nc.gpsimd.memset(ones16, BETA)

        # ---- persistent SBUF ----
        big = ctx.enter_context(tc.tile_pool(name="big", bufs=1))
        x8a_sb = big.tile([P, NQC, NCC, QC], F8, tag="x8a")
        x8b0_sb = big.tile([P, NCC, QC], F8, tag="x8b0")
        wq8_sb = big.tile([P, NCC, 2 * HPC * HD], F8, tag="wq8")
        wkv8_sb = big.tile([P, 4, NCC, HD], F8, tag="wkv8")
        kT16 = big.tile([P, T], F16, tag="kT16")
        kT016 = big.tile([P, QC], F16, tag="kT016")
        v8 = big.tile([P, NKT, HD], F8, tag="v8")
        v016 = big.tile([P, KTQ, HD], F16, tag="v016")

        # ---- pools ----
        qT_pool = ctx.enter_context(tc.tile_pool(name="qT", bufs=2))
        pt_pool = ctx.enter_context(tc.tile_pool(name="pt", bufs=6))
        pt16_pool = ctx.enter_context(tc.tile_pool(name="pt16", bufs=3))
        vt_pool = ctx.enter_context(tc.tile_pool(name="vt", bufs=2))
        vtt_pool = ctx.enter_context(tc.tile_pool(name="vtt", bufs=2))
        ysum_pool = ctx.enter_context(tc.tile_pool(name="ysum", bufs=10))
        ysumt_pool = ctx.enter_context(tc.tile_pool(name="ysumt", bufs=5))
        sums_sb_pool = ctx.enter_context(tc.tile_pool(name="ssb", bufs=3))
        sumst_pool = ctx.enter_context(tc.tile_pool(name="sumst", bufs=3))
        yout_pool = ctx.enter_context(tc.tile_pool(name="yout", bufs=2))
        recip_pool = ctx.enter_context(tc.tile_pool(name="recip", bufs=3))

        # PSUM (8 banks): st 2x[128,2,512] = 4, y 2x[128,512] = 2,
        # sums 2x[32,512] = 2. Projections share st slots.
        st_pp = ctx.enter_context(tc.tile_pool(name="st_pp", bufs=2,
                                               space="PSUM"))
        y_pp = ctx.enter_context(tc.tile_pool(name="y_pp", bufs=2,
                                              space="PSUM"))
        sums_pp = ctx.enter_context(tc.tile_pool(name="sums_pp", bufs=2,
                                                 space="PSUM"))

        # ---- input DMAs: one queue, priority order, so the startup-critical
        # transfers (wkv8 + x chunk 0) are not bandwidth-starved ----
        xr = x8a.rearrange("(cc p) t -> p cc t", p=P)
        wkv_r = wkv8.rearrange("p (f cc d) -> p f cc d", f=4, cc=NCC)
        nc.sync.dma_start(out=wkv8_sb[:, 0], in_=wkv_r[:, 0])  # wk8a first
        nc.sync.dma_start(out=x8a_sb[:, 0, 0:NCC // 2], in_=xr[:, 0:NCC // 2, 0:QC])
        nc.sync.dma_start(out=x8a_sb[:, 0, NCC // 2:], in_=xr[:, NCC // 2:, 0:QC])
        nc.sync.dma_start(out=wkv8_sb[:, 1:], in_=wkv_r[:, 1:])
        nc.sync.dma_start(out=x8b0_sb,
                          in_=x8b0.rearrange("(cc p) t -> p cc t", p=P))
        nc.sync.dma_start(out=wq8_sb,
                          in_=wq8.rearrange("p (cc d) -> p cc d", cc=NCC))
        for tq in range(1, NQC):
            nc.sync.dma_start(out=x8a_sb[:, tq],
                              in_=xr[:, :, tq * QC:(tq + 1) * QC])

        def dr_proj(ps, w_sl, x_sl, first, last):
            # 8 DoubleRow matmuls: contraction C in pairs of 128-row tiles
            for c4 in range(NCC // 2):
                nc.tensor.matmul(
                    ps, lhsT=w_sl(c4), rhs=x_sl(c4),
                    start=(first and c4 == 0), stop=(last and c4 == NCC // 2 - 1),
                    perf_mode=DR)

        def wkv_slice(f):
            return lambda c4: wkv8_sb[:, f, 2 * c4:2 * c4 + 2]

        def wq_slice(term, h):
            off = term * HPC * HD + h * HD
            return lambda c4: wq8_sb[:, 2 * c4:2 * c4 + 2, off:off + HD]

        def x_slice(tq):
            return lambda c4: x8a_sb[:, tq, 2 * c4:2 * c4 + 2]

        def x0b_slice():
            return lambda c4: x8b0_sb[:, 2 * c4:2 * c4 + 2]

        def k_chunk(tq):
            ps = st_pp.tile([P, QC], F32, tag="st")
            dr_proj(ps, wkv_slice(0), x_slice(tq), True, True)
            nc.vector.tensor_copy(kT16[:, tq * QC:(tq + 1) * QC], ps)

        def v_chunk(tq, two_term):
            ps = st_pp.tile([P, QC], F32, tag="st")
            dr_proj(ps, wkv_slice(2), x_slice(tq), True, not two_term)
            if two_term:
                dr_proj(ps, wkv_slice(3), x_slice(tq), False, False)
                dr_proj(ps, wkv_slice(2), x0b_slice(), False, True)
            vt16 = vt_pool.tile([P, QC], F16, tag="vt")
            nc.vector.tensor_copy(vt16, ps)
            if two_term:
                nc.sync.dma_start_transpose(v016, vt16)
            else:
                vtt = vtt_pool.tile([P, KTQ, HD], F16, tag="vtt")
                nc.sync.dma_start_transpose(vtt, vt16)
                nc.vector.tensor_copy(v8[:, tq * KTQ:(tq + 1) * KTQ], vtt)

        with nc.named_scope("kv0"):
            _mark(nc, "kv0")
            ps0 = st_pp.tile([P, QC], F32, tag="st")
            dr_proj(ps0, wkv_slice(0), x_slice(0), True, False)
            dr_proj(ps0, wkv_slice(1), x_slice(0), False, False)
            dr_proj(ps0, wkv_slice(0), x0b_slice(), False, True)
            nc.vector.tensor_copy(kT016, ps0)
            v_chunk(0, True)

        # ---- Q projection per (chunk, head) ----
        qts = {}

        def emit_qproj(tq):
            _mark(nc, f"qproj{tq}")
            with nc.named_scope(f"qproj{tq}"):
                qTq = qT_pool.tile([P, HPC, QC], F16, tag="qT")
                for h in range(HPC):
                    ps = st_pp.tile([P, QC], F32, tag="st")
                    dr_proj(ps, wq_slice(0, h), x_slice(tq), True, tq != 0)
                    if tq == 0:  # three-term q for the low-key-count chunk
                        dr_proj(ps, wq_slice(1, h), x_slice(0), False, False)
                        dr_proj(ps, wq_slice(0, h), x0b_slice(), False, True)
                    nc.vector.tensor_copy(qTq[:, h], ps)
                qts[tq] = qTq

        pending_tails = []
        chunk_tail_state = {}

        def emit_tail(tq, hp, ysum2, sums2):
            # Per head-pair, deferred one chunk. PE-free: transposes via the
            # DMA XBAR. One sums transpose + one output DMA per chunk.
            with nc.named_scope(f"ltail{tq}p{hp}"):
                _mark(nc, f"q{tq}:ltail{hp}")
                if tq not in chunk_tail_state:
                    yo = yout_pool.tile([P, KTQ, HPC, HD], F16, tag="yo")
                    chunk_tail_state[tq] = yo
                yo = chunk_tail_state[tq]
                sumst = sumst_pool.tile([P, KTQ, 64], F16, tag="sumst")
                nc.sync.dma_start_transpose(
                    sumst, sums2[hp * 64:hp * 64 + 64, :])
                rt = recip_pool.tile([P, KTQ, 2], F32, tag="recip")
                for i in range(2):
                    nc.vector.reciprocal(rt[:, :, i:i + 1],
                                         sumst[:, :, i * 32:i * 32 + 1])
                yst = ysumt_pool.tile([P, 2 * KTQ, HD], F16, tag="yst")
                nc.scalar.dma_start_transpose(
                    yst, ysum2.rearrange("p two q -> p (two q)"))
                for i in range(2):
                    for qt in range(KTQ):
                        nc.vector.tensor_scalar_mul(
                            yo[:, qt, 2 * hp + i], yst[:, i * KTQ + qt],
                            rt[:, qt, i:i + 1])
                if hp == 1:
                    ydst = y.rearrange("(nq qt p) (hh d) -> nq p qt hh d",
                                       qt=KTQ, p=P, hh=HPC)[tq]
                    nc.sync.dma_start(out=ydst, in_=yo)

        def emit_attn(tq):
            # ---- causal attention for this query chunk ----
            # Head PAIRS: the PE alternates score and PV matmuls between the
            # two heads so each head's exp chain hides under the other's
            # matmuls. Chunk 0 uses the accurate fp16 path.
            last_chunk = tq == NQC - 1
            fp16_path = tq == 0
            nkt = (tq + 1) * KTQ
            ngr = nkt // 2  # key-tile pairs
            qTq = qts.pop(tq)
            sums2 = sums_sb_pool.tile([P, QC], F16, tag="ssb")
            kt_src = kT016 if fp16_path else kT16

            def s_mm(h, g):
                # scores + exp (+ diag mask) for key-tile pair g
                st = st_pp.tile([P, 2, QC], F32, tag="st")
                if fp16_path:
                    pt = pt16_pool.tile([P, 2, QC], F16, tag="pt16")
                else:
                    pt = pt_pool.tile([P, 2, QC], F8, tag="pt")
                qrhs = qTq[:, h]
                offs = []
                for u in range(2):
                    kt = 2 * g + u
                    off = max((kt - KTQ * tq) * P, 0)
                    offs.append(off)
                    nc.tensor.matmul(
                        st[:, u, off:],
                        lhsT=kt_src[:, kt * P:(kt + 1) * P],
                        rhs=qrhs[:, off:], start=True, stop=True)
                o0, o1 = offs
                if 2 * g + 1 >= KTQ * tq:  # pair contains diagonal tiles
                    # split exp per tile so the Pool mask of u0 overlaps the
                    # exp of u1 instead of serializing behind one big exp
                    for u, off in enumerate(offs):
                        nc.scalar.activation(pt[:, u, off:], st[:, u, off:],
                                             mybir.ActivationFunctionType.Exp,
                                             scale=exp_scale)
                        nc.gpsimd.affine_select(
                            out=pt[:, u, off:off + P],
                            in_=pt[:, u, off:off + P],
                            pattern=[[1, P]],
                            compare_op=mybir.AluOpType.is_ge,
                            fill=0.0, base=0, channel_multiplier=-1)
                    if o1 > o0:
                        nc.gpsimd.memset(pt[:, 1, o0:o1], 0.0)
                else:
                    nc.scalar.activation(pt[:, :, o0:], st[:, :, o0:],
                                         mybir.ActivationFunctionType.Exp,
                                         scale=exp_scale)
                return pt, o0

            for hp in range(HPC // 2):
              pair = (2 * hp, 2 * hp + 1)
              with nc.named_scope(f"attn{tq}p{hp}"):
                y_psh = {}
                s_psh = {}
                pts = {}
                for h in pair:
                    y_ps = y_pp.tile([P, QC], F32, tag="y")
                    y_psh[h] = y_ps
                    s_ps = sums_pp.tile([32, QC], F32, tag="sums")
                    s_psh[h] = s_ps
                    pts[h] = {0: s_mm(h, 0)}
                for g in range(ngr):
                    _mark(nc, f"q{tq}:att{hp}g{g}")
                    if g + 1 < ngr:
                        for h in pair:
                            pts[h][g + 1] = s_mm(h, g + 1)
                    for h in pair:
                        pt, o0 = pts[h].pop(g)
                        first, last = g == 0, g == ngr - 1
                        if fp16_path:
                            for u in range(2):
                                kt = 2 * g + u
                                off = max((kt - KTQ * tq) * P, 0)
                                nc.tensor.matmul(
                                    y_psh[h][:, off:], lhsT=v016[:, kt],
                                    rhs=pt[:, u, off:],
                                    start=(kt == 0), stop=(kt == nkt - 1),
                                    skip_group_check=True)
                                nc.tensor.matmul(
                                    s_psh[h][:, off:], lhsT=ones16,
                                    rhs=pt[:, u, off:],
                                    start=(kt == 0), stop=(kt == nkt - 1),
                                    skip_group_check=True)
                        else:
                            v_sl = v8.rearrange("p (gg two) d -> p gg two d",
                                                two=2)[:, g]
                            nc.tensor.matmul(
                                y_psh[h][:, o0:], lhsT=v_sl,
                                rhs=pt[:, :, o0:],
                                start=first, stop=last, perf_mode=DR,
                                skip_group_check=True)
                            nc.tensor.matmul(
                                s_psh[h][:, o0:], lhsT=ones8,
                                rhs=pt[:, :, o0:],
                                start=first, stop=last, perf_mode=DR,
                                skip_group_check=True)
                _mark(nc, f"q{tq}:tail{hp}")
                ysum2 = ysum_pool.tile([P, 2, QC], F16, tag="ysum")
                for i, h in enumerate(pair):
                    nc.vector.tensor_scalar_mul(ysum2[:, i], y_psh[h], ALPHA)
                    nc.vector.tensor_copy(sums2[h * 32:h * 32 + 1, :],
                                          s_psh[h][0:1, :])
                pending_tails.append((tq, hp, ysum2, sums2))
                # keep <= 2 pending (pipelines tails under the next chunk's
                # matmuls); on the last chunk drain eagerly so each tail
                # overlaps the remaining pairs' matmuls
                limit = 0 if last_chunk else 2
                while len(pending_tails) > limit:
                    emit_tail(*pending_tails.pop(0))

        # ---- emission order: minimal work before attn0 (K0s/V0s are only
        # needed from attn1 on, so they ride with kv1) ----
        emit_qproj(0)
        emit_attn(0)
        for tq in range(1, NQC):
            with nc.named_scope(f"kv{tq}"):
                _mark(nc, f"kv{tq}")
                if tq == 1:
                    k_chunk(0)
                    v_chunk(0, False)
                k_chunk(tq)
                v_chunk(tq, False)
            emit_qproj(tq)
            emit_attn(tq)

    nc.compile()
    return nc


_cache = {}


def _get_nc(T, C):
    key = (T, C)
    if key not in _cache:
        _cache[key] = build_nc(T, C)
    return _cache[key]


def prepare_in_maps(x, w_kv, w_q):
    x = np.asarray(x, dtype=np.float32)
    w_kv = np.asarray(w_kv, dtype=np.float32)
    w_q = np.asarray(w_q, dtype=np.float32)
    B, T, C = x.shape

    NCC = C // P

    def two_term(w):  # [C, D] scaled two-term fp8
        ws = np.ascontiguousarray(w) * S
        a = ws.astype(E4M3)
        b = (ws - a.astype(np.float32)).astype(E4M3)
        return a, b

    def shuffle(w8):  # [C, D] -> [P, NCC*D] matching [p, cc, d] tiles
        D = w8.shape[1]
        return w8.reshape(NCC, P, D).transpose(1, 0, 2).reshape(P, NCC * D)

    wk = w_kv[:HD].T  # [C, HD]
    wv = w_kv[HD:].T
    wk8a, wk8b = two_term(wk)
    wv8a, wv8b = two_term(wv)
    wkv8 = np.ascontiguousarray(np.concatenate(
        [shuffle(w) for w in (wk8a, wk8b, wv8a, wv8b)], axis=1))

    in_maps = []
    for i in range(N_CORES):
        b, hg = divmod(i, NB)
        xT = np.ascontiguousarray(x[b].T)  # [C, T]
        x8a = xT.astype(E4M3)
        x8b0 = (xT[:, :QC] - x8a[:, :QC].astype(np.float32)).astype(E4M3)
        wqs = w_q[hg * HPC * HD:(hg + 1) * HPC * HD].T  # [C, 512]
        wq8a, wq8b = two_term(wqs)
        # interleave per-cc: [P, NCC, 2*512] flattened
        sa = shuffle(wq8a).reshape(P, NCC, HPC * HD)
        sb = shuffle(wq8b).reshape(P, NCC, HPC * HD)
        wq8 = np.ascontiguousarray(
            np.concatenate([sa, sb], axis=2).reshape(P, NCC * 2 * HPC * HD))
        in_maps.append({"x8a": x8a, "x8b0": np.ascontiguousarray(x8b0),
                        "wq8": wq8, "wkv8": wkv8})
    return in_maps


def gather_output(results, B, T, C):
    out = np.empty((B, T, C), np.float32)
    for i in range(N_CORES):
        b, hg = divmod(i, NB)
        out[b, :, hg * HPC * HD:(hg + 1) * HPC * HD] = \
            results[i]["y"].astype(np.float32)
    return out


def kernel(x, w_kv, w_q):
    x = np.asarray(x)
    B, T, C = x.shape
    nc = _get_nc(T, C)
    in_maps = prepare_in_maps(x, w_kv, w_q)
    res = run_bass_kernel_spmd(nc, in_maps, list(range(N_CORES)))
    return gather_output(res.results, B, T, C)


# revision 26
# speedup vs baseline: 1.0191x; 1.0157x over previous
"""Causal MQA kernel for Trainium2, SPMD over 8 NeuronCores.

Sharding: core i = (batch b = i//4, head-group hg = i%4). Each core computes
K/V projections for its batch locally (no collectives), the q projection for
its 4 heads, and causal attention for those heads; it writes the [T, 512]
fp16 output slice y[b, :, hg*512:(hg+1)*512]. The host concatenates slices.

Device algorithm (per core, T processed in 4 chunks of QC=512 queries):
  - Projections run in fp8e4 with DoubleRow perf mode (2 contraction k-tiles
    per instruction, 2x PE throughput): weights are scaled by S=32 on the
    host to avoid fp8 subnormals; the S^2 factor folds into the exp scale
    and the S factor on V folds into the softmax-denominator constant.
  - Scores S^T[k, q] = matmul(lhsT=kT16 tile, rhs=qT16 chunk) in fp16.
  - P^T = exp(S^T * scale) emitted directly to fp8 (ACT), one instruction
    per key-tile pair; causal masking on diagonal tiles via gpsimd
    affine_select on the [128,128] triangle + a gap memset (both on Pool).
  - PV: y^T accumulates in PSUM via fp8 DoubleRow over key-tile pairs
    (lhsT = v8 pair, rhs = pt8 pair).
  - Softmax denominators: fp8 DoubleRow ones-matmul (lhsT = const 0.5) into
    a [32, 512] PSUM accumulator per head, accumulated across pairs.
  - fp8 error control: chunk 0 (queries 0-511, keys 0-511) runs an
    accurate path -- three-term q/k/v projections (~0.5% error) with fp16
    scores/pt16/PV -- because its low-key-count rows get no softmax
    averaging; later rows tolerate the fp8 noise (measured ~1.5e-2 max).
  - All transposes (v, denominators, y^T) go through the DMA XBAR
    (dma_start_transpose), keeping the PE free; the tail is deferred one
    chunk and does reciprocal + per-partition scale-multiply on DVE.
"""

import math
from contextlib import ExitStack

import numpy as np
import ml_dtypes

import concourse.bass as bass
import concourse.mybir as mybir
import concourse.tile as tile
from concourse import bacc
from concourse.bass_utils import run_bass_kernel_spmd

F32 = mybir.dt.float32
F16 = mybir.dt.float16
F8 = mybir.dt.float8e4
E4M3 = ml_dtypes.float8_e4m3

P = 128  # partitions
HD = 128  # head dim
QC = 512  # query-chunk width (one fp32 PSUM bank)
N_CORES = 8
HPC = 4  # query heads per core
NB = 4  # head groups (cores per batch)
S = 32.0  # host-side weight scale (fp8 subnormal avoidance)
ALPHA = 1.0 / 64.0  # ysum copy scale (fp16 overflow avoidance)
BETA = S * ALPHA  # denominator matmul constant = 0.5 (exact in fp8)
DR = mybir.MatmulPerfMode.DoubleRow

PHASE_MARKS = []


def _mark(nc, name):
    n = int(nc.get_next_instruction_name().split("-")[-1])
    PHASE_MARKS.append((n, name))


def build_nc(T, C):
    NQC = T // QC  # query chunks (4)
    NCC = C // P  # contraction chunks (16)
    KTQ = QC // P  # key tiles per query chunk (4)
    NKT = T // P  # key tiles total (16)
    exp_scale = 1.0 / (math.sqrt(HD) * S * S)

    nc = bacc.Bacc("TRN2", target_bir_lowering=False, debug=False,
                   num_devices=N_CORES)
    x8a = nc.dram_tensor("x8a", [C, T], F8, kind="ExternalInput").ap()
    x8b0 = nc.dram_tensor("x8b0", [C, QC], F8, kind="ExternalInput").ap()
    # pre-shuffled on host to [P, ...] so weight DMAs are contiguous copies
    NCC_ = C // P
    wq8 = nc.dram_tensor("wq8", [P, NCC_ * 2 * HPC * HD], F8,
                         kind="ExternalInput").ap()
    # wkv8 = [wk8a | wk8b | wv8a | wv8b], each [P, NCC*HD]
    wkv8 = nc.dram_tensor("wkv8", [P, 4 * NCC_ * HD], F8,
                          kind="ExternalInput").ap()
    y = nc.dram_tensor("y", [T, HPC * HD], F16, kind="ExternalOutput").ap()

    with tile.TileContext(nc) as tc, ExitStack() as ctx, \
            nc.allow_low_precision(reason="fp8 operands feed the PE; accumulation stays fp32 in PSUM"):
        consts = ctx.enter_context(tc.tile_pool(name="consts", bufs=1))
        ones8 = consts.tile([P, 2, 32], F8, tag="ones8")
        nc.gpsimd.memset(ones8, BETA)
        ones16 = consts.tile([P, 32], F16, tag="ones16")
        nc.gpsimd.memset(ones16, BETA)

        # ---- persistent SBUF ----
        big = ctx.enter_context(tc.tile_pool(name="big", bufs=1))
        x8a_sb = big.tile([P, NQC, NCC, QC], F8, tag="x8a")
        x8b0_sb = big.tile([P, NCC, QC], F8, tag="x8b0")
        wq8_sb = big.tile([P, NCC, 2 * HPC * HD], F8, tag="wq8")
        wkv8_sb = big.tile([P, 4, NCC, HD], F8, tag="wkv8")
        kT16 = big.tile([P, T], F16, tag="kT16")
        kT016 = big.tile([P, QC], F16, tag="kT016")
        v8 = big.tile([P, NKT, HD], F8, tag="v8")
        v016 = big.tile([P, KTQ, HD], F16, tag="v016")

        # ---- pools ----
        qT_pool = ctx.enter_context(tc.tile_pool(name="qT", bufs=2))
        pt_pool = ctx.enter_context(tc.tile_pool(name="pt", bufs=6))
        pt16_pool = ctx.enter_context(tc.tile_pool(name="pt16", bufs=3))
        vt_pool = ctx.enter_context(tc.tile_pool(name="vt", bufs=2))
        vtt_pool = ctx.enter_context(tc.tile_pool(name="vtt", bufs=2))
        ysum_pool = ctx.enter_context(tc.tile_pool(name="ysum", bufs=10))
        ysumt_pool = ctx.enter_context(tc.tile_pool(name="ysumt", bufs=5))
        sums_sb_pool = ctx.enter_context(tc.tile_pool(name="ssb", bufs=3))
        sumst_pool = ctx.enter_context(tc.tile_pool(name="sumst", bufs=3))
        yout_pool = ctx.enter_context(tc.tile_pool(name="yout", bufs=2))
        recip_pool = ctx.enter_context(tc.tile_pool(name="recip", bufs=3))

        # PSUM (8 banks): st 2x[128,2,512] = 4, y 2x[128,512] = 2,
        # sums 2x[32,512] = 2. Projections share st slots.
        st_pp = ctx.enter_context(tc.tile_pool(name="st_pp", bufs=2,
                                               space="PSUM"))
        y_pp = ctx.enter_context(tc.tile_pool(name="y_pp", bufs=2,
                                              space="PSUM"))
        sums_pp = ctx.enter_context(tc.tile_pool(name="sums_pp", bufs=2,
                                                 space="PSUM"))

        # ---- input DMAs: one queue, priority order, so the startup-critical
        # transfers (wkv8 + x chunk 0) are not bandwidth-starved ----
        xr = x8a.rearrange("(cc p) t -> p cc t", p=P)
        wkv_r = wkv8.rearrange("p (f cc d) -> p f cc d", f=4, cc=NCC)
        nc.sync.dma_start(out=wkv8_sb[:, 0], in_=wkv_r[:, 0])  # wk8a first
        nc.sync.dma_start(out=x8a_sb[:, 0, 0:NCC // 2], in_=xr[:, 0:NCC // 2, 0:QC])
        nc.sync.dma_start(out=x8a_sb[:, 0, NCC // 2:], in_=xr[:, NCC // 2:, 0:QC])
        nc.sync.dma_start(out=wkv8_sb[:, 1:], in_=wkv_r[:, 1:])
        nc.sync.dma_start(out=x8b0_sb,
                          in_=x8b0.rearrange("(cc p) t -> p cc t", p=P))
        nc.sync.dma_start(out=wq8_sb,
                          in_=wq8.rearrange("p (cc d) -> p cc d", cc=NCC))
        for tq in range(1, NQC):
            nc.sync.dma_start(out=x8a_sb[:, tq],
                              in_=xr[:, :, tq * QC:(tq + 1) * QC])

        def dr_proj(ps, w_sl, x_sl, first, last):
            # 8 DoubleRow matmuls: contraction C in pairs of 128-row tiles
            for c4 in range(NCC // 2):
                nc.tensor.matmul(
                    ps, lhsT=w_sl(c4), rhs=x_sl(c4),
                    start=(first and c4 == 0), stop=(last and c4 == NCC // 2 - 1),
                    perf_mode=DR)

        def wkv_slice(f):
            return lambda c4: wkv8_sb[:, f, 2 * c4:2 * c4 + 2]

        def wq_slice(term, h):
            off = term * HPC * HD + h * HD
            return lambda c4: wq8_sb[:, 2 * c4:2 * c4 + 2, off:off + HD]

        def x_slice(tq):
            return lambda c4: x8a_sb[:, tq, 2 * c4:2 * c4 + 2]

        def x0b_slice():
            return lambda c4: x8b0_sb[:, 2 * c4:2 * c4 + 2]

        def k_chunk(tq):
            ps = st_pp.tile([P, QC], F32, tag="st")
            dr_proj(ps, wkv_slice(0), x_slice(tq), True, True)
            nc.vector.tensor_copy(kT16[:, tq * QC:(tq + 1) * QC], ps)

        def v_chunk(tq, two_term):
            ps = st_pp.tile([P, QC], F32, tag="st")
            dr_proj(ps, wkv_slice(2), x_slice(tq), True, not two_term)
            if two_term:
                dr_proj(ps, wkv_slice(3), x_slice(tq), False, False)
                dr_proj(ps, wkv_slice(2), x0b_slice(), False, True)
            vt16 = vt_pool.tile([P, QC], F16, tag="vt")
            nc.vector.tensor_copy(vt16, ps)
            if two_term:
                nc.sync.dma_start_transpose(v016, vt16)
            else:
                vtt = vtt_pool.tile([P, KTQ, HD], F16, tag="vtt")
                nc.sync.dma_start_transpose(vtt, vt16)
                nc.vector.tensor_copy(v8[:, tq * KTQ:(tq + 1) * KTQ], vtt)

        with nc.named_scope("kv0"):
            _mark(nc, "kv0")
            ps0 = st_pp.tile([P, QC], F32, tag="st")
            dr_proj(ps0, wkv_slice(0), x_slice(0), True, False)
            dr_proj(ps0, wkv_slice(1), x_slice(0), False, False)
            dr_proj(ps0, wkv_slice(0), x0b_slice(), False, True)
            nc.vector.tensor_copy(kT016, ps0)
            v_chunk(0, True)

        # ---- Q projection per (chunk, head) ----
        qts = {}

        def emit_qproj(tq):
            _mark(nc, f"qproj{tq}")
            with nc.named_scope(f"qproj{tq}"):
                qTq = qT_pool.tile([P, HPC, QC], F16, tag="qT")
                for h in range(HPC):
                    ps = st_pp.tile([P, QC], F32, tag="st")
                    dr_proj(ps, wq_slice(0, h), x_slice(tq), True, tq != 0)
                    if tq == 0:  # three-term q for the low-key-count chunk
                        dr_proj(ps, wq_slice(1, h), x_slice(0), False, False)
                        dr_proj(ps, wq_slice(0, h), x0b_slice(), False, True)
                    nc.vector.tensor_copy(qTq[:, h], ps)
                qts[tq] = qTq

        pending_tails = []
        chunk_tail_state = {}

        def emit_tail(tq, hp, ysum2, sums2):
            # Per head-pair, deferred one chunk. PE-free: transposes via the
            # DMA XBAR. One sums transpose + one output DMA per chunk.
            with nc.named_scope(f"ltail{tq}p{hp}"):
                _mark(nc, f"q{tq}:ltail{hp}")
                if tq not in chunk_tail_state:
                    yo = yout_pool.tile([P, KTQ, HPC, HD], F16, tag="yo")
                    chunk_tail_state[tq] = yo
                yo = chunk_tail_state[tq]
                sumst = sumst_pool.tile([P, KTQ, 64], F16, tag="sumst")
                nc.sync.dma_start_transpose(
                    sumst, sums2[hp * 64:hp * 64 + 64, :])
                rt = recip_pool.tile([P, KTQ, 2], F32, tag="recip")
                for i in range(2):
                    nc.vector.reciprocal(rt[:, :, i:i + 1],
                                         sumst[:, :, i * 32:i * 32 + 1])
                yst = ysumt_pool.tile([P, 2 * KTQ, HD], F16, tag="yst")
                nc.scalar.dma_start_transpose(
                    yst, ysum2.rearrange("p two q -> p (two q)"))
                for i in range(2):
                    for qt in range(KTQ):
                        nc.vector.tensor_scalar_mul(
                            yo[:, qt, 2 * hp + i], yst[:, i * KTQ + qt],
                            rt[:, qt, i:i + 1])
                if hp == 1:
                    ydst = y.rearrange("(nq qt p) (hh d) -> nq p qt hh d",
                                       qt=KTQ, p=P, hh=HPC)[tq]
                    nc.sync.dma_start(out=ydst, in_=yo)

        def emit_attn(tq):
            # ---- causal attention for this query chunk ----
            # Head PAIRS: the PE alternates score and PV matmuls between the
            # two heads so each head's exp chain hides under the other's
            # matmuls. Chunk 0 uses the accurate fp16 path.
            last_chunk = tq == NQC - 1
            fp16_path = tq == 0
            nkt = (tq + 1) * KTQ
            ngr = nkt // 2  # key-tile pairs
            qTq = qts.pop(tq)
            sums2 = sums_sb_pool.tile([P, QC], F16, tag="ssb")
            kt_src = kT016 if fp16_path else kT16

            def s_mm(h, g):
                # scores + exp (+ diag mask) for key-tile pair g
                st = st_pp.tile([P, 2, QC], F32, tag="st")
                if fp16_path:
                    pt = pt16_pool.tile([P, 2, QC], F16, tag="pt16")
                else:
                    pt = pt_pool.tile([P, 2, QC], F8, tag="pt")
                qrhs = qTq[:, h]
                offs = []
                for u in range(2):
                    kt = 2 * g + u
                    off = max((kt - KTQ * tq) * P, 0)
                    offs.append(off)
                    nc.tensor.matmul(
                        st[:, u, off:],
                        lhsT=kt_src[:, kt * P:(kt + 1) * P],
                        rhs=qrhs[:, off:], start=True, stop=True)
                o0, o1 = offs
                nc.scalar.activation(pt[:, :, o0:], st[:, :, o0:],
                                     mybir.ActivationFunctionType.Exp,
                                     scale=exp_scale)
                if 2 * g + 1 >= KTQ * tq:  # pair contains diagonal tiles
                    if o1 > o0:
                        nc.gpsimd.memset(pt[:, 1, o0:o1], 0.0)
                    for u, off in enumerate(offs):
                        nc.gpsimd.affine_select(
                            out=pt[:, u, off:off + P],
                            in_=pt[:, u, off:off + P],
                            pattern=[[1, P]],
                            compare_op=mybir.AluOpType.is_ge,
                            fill=0.0, base=0, channel_multiplier=-1)
                return pt, o0

            for hp in range(HPC // 2):
              pair = (2 * hp, 2 * hp + 1)
              with nc.named_scope(f"attn{tq}p{hp}"):
                y_psh = {}
                s_psh = {}
                pts = {}
                for h in pair:
                    y_ps = y_pp.tile([P, QC], F32, tag="y")
                    y_psh[h] = y_ps
                    s_ps = sums_pp.tile([32, QC], F32, tag="sums")
                    s_psh[h] = s_ps
                    pts[h] = {0: s_mm(h, 0)}
                for g in range(ngr):
                    _mark(nc, f"q{tq}:att{hp}g{g}")
                    if g + 1 < ngr:
                        for h in pair:
                            pts[h][g + 1] = s_mm(h, g + 1)
                    for h in pair:
                        pt, o0 = pts[h].pop(g)
                        first, last = g == 0, g == ngr - 1
                        if fp16_path:
                            for u in range(2):
                                kt = 2 * g + u
                                off = max((kt - KTQ * tq) * P, 0)
                                nc.tensor.matmul(
                                    y_psh[h][:, off:], lhsT=v016[:, kt],
                                    rhs=pt[:, u, off:],
                                    start=(kt == 0), stop=(kt == nkt - 1),
                                    skip_group_check=True)
                                nc.tensor.matmul(
                                    s_psh[h][:, off:], lhsT=ones16,
                                    rhs=pt[:, u, off:],
                                    start=(kt == 0), stop=(kt == nkt - 1),
                                    skip_group_check=True)
                        else:
                            v_sl = v8.rearrange("p (gg two) d -> p gg two d",
                                                two=2)[:, g]
                            nc.tensor.matmul(
                                y_psh[h][:, o0:], lhsT=v_sl,
                                rhs=pt[:, :, o0:],
                                start=first, stop=last, perf_mode=DR,
                                skip_group_check=True)
                            nc.tensor.matmul(
                                s_psh[h][:, o0:], lhsT=ones8,
                                rhs=pt[:, :, o0:],
                                start=first, stop=last, perf_mode=DR,
                                skip_group_check=True)
                _mark(nc, f"q{tq}:tail{hp}")
                ysum2 = ysum_pool.tile([P, 2, QC], F16, tag="ysum")
                for i, h in enumerate(pair):
                    nc.vector.tensor_scalar_mul(ysum2[:, i], y_psh[h], ALPHA)
                    nc.vector.tensor_copy(sums2[h * 32:h * 32 + 1, :],
                                          s_psh[h][0:1, :])
                pending_tails.append((tq, hp, ysum2, sums2))
                # keep <= 2 pending (pipelines tails under the next chunk's
                # matmuls); on the last chunk drain eagerly so each tail
                # overlaps the remaining pairs' matmuls
                limit = 0 if last_chunk else 2
                while len(pending_tails) > limit:
                    emit_tail(*pending_tails.pop(0))

        # ---- emission order: minimal work before attn0 (K0s/V0s are only
        # needed from attn1 on, so they ride with kv1) ----
        emit_qproj(0)
        emit_attn(0)
        for tq in range(1, NQC):
            with nc.named_scope(f"kv{tq}"):
                _mark(nc, f"kv{tq}")
                if tq == 1:
                    k_chunk(0)
                    v_chunk(0, False)
                k_chunk(tq)
                v_chunk(tq, False)
            emit_qproj(tq)
            emit_attn(tq)

    nc.compile()
    return nc


_cache = {}


def _get_nc(T, C):
    key = (T, C)
    if key not in _cache:
        _cache[key] = build_nc(T, C)
    return _cache[key]


def prepare_in_maps(x, w_kv, w_q):
    x = np.asarray(x, dtype=np.float32)
    w_kv = np.asarray(w_kv, dtype=np.float32)
    w_q = np.asarray(w_q, dtype=np.float32)
    B, T, C = x.shape

    NCC = C // P

    def two_term(w):  # [C, D] scaled two-term fp8
        ws = np.ascontiguousarray(w) * S
        a = ws.astype(E4M3)
        b = (ws - a.astype(np.float32)).astype(E4M3)
        return a, b

    def shuffle(w8):  # [C, D] -> [P, NCC*D] matching [p, cc, d] tiles
        D = w8.shape[1]
        return w8.reshape(NCC, P, D).transpose(1, 0, 2).reshape(P, NCC * D)

    wk = w_kv[:HD].T  # [C, HD]
    wv = w_kv[HD:].T
    wk8a, wk8b = two_term(wk)
    wv8a, wv8b = two_term(wv)
    wkv8 = np.ascontiguousarray(np.concatenate(
        [shuffle(w) for w in (wk8a, wk8b, wv8a, wv8b)], axis=1))

    in_maps = []
    for i in range(N_CORES):
        b, hg = divmod(i, NB)
        xT = np.ascontiguousarray(x[b].T)  # [C, T]
        x8a = xT.astype(E4M3)
        x8b0 = (xT[:, :QC] - x8a[:, :QC].astype(np.float32)).astype(E4M3)
        wqs = w_q[hg * HPC * HD:(hg + 1) * HPC * HD].T  # [C, 512]
        wq8a, wq8b = two_term(wqs)
        # interleave per-cc: [P, NCC, 2*512] flattened
        sa = shuffle(wq8a).reshape(P, NCC, HPC * HD)
        sb = shuffle(wq8b).reshape(P, NCC, HPC * HD)
        wq8 = np.ascontiguousarray(
            np.concatenate([sa, sb], axis=2).reshape(P, NCC * 2 * HPC * HD))
        in_maps.append({"x8a": x8a, "x8b0": np.ascontiguousarray(x8b0),
                        "wq8": wq8, "wkv8": wkv8})
    return in_maps


def gather_output(results, B, T, C):
    out = np.empty((B, T, C), np.float32)
    for i in range(N_CORES):
        b, hg = divmod(i, NB)
        out[b, :, hg * HPC * HD:(hg + 1) * HPC * HD] = \
            results[i]["y"].astype(np.float32)
    return out


def kernel(x, w_kv, w_q):
    x = np.asarray(x)
    B, T, C = x.shape
    nc = _get_nc(T, C)
    in_maps = prepare_in_maps(x, w_kv, w_q)
    res = run_bass_kernel_spmd(nc, in_maps, list(range(N_CORES)))
    return gather_output(res.results, B, T, C)


# revision 28
# speedup vs baseline: 1.0257x; 1.0065x over previous
"""Causal MQA kernel for Trainium2, SPMD over 8 NeuronCores.

Sharding: core i = (batch b = i//4, head-group hg = i%4). Each core computes
K/V projections for its batch locally (no collectives), the q projection for
its 4 heads, and causal attention for those heads; it writes the [T, 512]
fp16 output slice y[b, :, hg*512:(hg+1)*512]. The host concatenates slices.

Device algorithm (per core, T processed in 4 chunks of QC=512 queries):
  - Projections run in fp8e4 with DoubleRow perf mode (2 contraction k-tiles
    per instruction, 2x PE throughput): weights are scaled by S=32 on the
    host to avoid fp8 subnormals; the S^2 factor folds into the exp scale
    and the S factor on V folds into the softmax-denominator constant.
  - Scores S^T[k, q] = matmul(lhsT=kT16 tile, rhs=qT16 chunk) in fp16.
  - P^T = exp(S^T * scale) emitted directly to fp8 (ACT), one instruction
    per key-tile pair; causal masking on diagonal tiles via gpsimd
    affine_select on the [128,128] triangle + a gap memset (both on Pool).
  - PV: y^T accumulates in PSUM via fp8 DoubleRow over key-tile pairs
    (lhsT = v8 pair, rhs = pt8 pair).
  - Softmax denominators: fp8 DoubleRow ones-matmul (lhsT = const 0.5) into
    a [32, 512] PSUM accumulator per head, accumulated across pairs.
  - fp8 error control: chunk 0 (queries 0-511, keys 0-511) runs an
    accurate path -- three-term q/k/v projections (~0.5% error) with fp16
    scores/pt16/PV -- because its low-key-count rows get no softmax
    averaging; later rows tolerate the fp8 noise (measured ~1.5e-2 max).
  - All transposes (v, denominators, y^T) go through the DMA XBAR
    (dma_start_transpose), keeping the PE free; the tail is deferred one
    chunk and does reciprocal + per-partition scale-multiply on DVE.
"""

import math
from contextlib import ExitStack

import numpy as np
import ml_dtypes

import concourse.bass as bass
import concourse.mybir as mybir
import concourse.tile as tile
from concourse import bacc
from concourse.bass_utils import run_bass_kernel_spmd

F32 = mybir.dt.float32
F16 = mybir.dt.float16
F8 = mybir.dt.float8e4
E4M3 = ml_dtypes.float8_e4m3

P = 128  # partitions
HD = 128  # head dim
QC = 512  # query-chunk width (one fp32 PSUM bank)
N_CORES = 8
HPC = 4  # query heads per core
NB = 4  # head groups (cores per batch)
S = 32.0  # host-side weight scale (fp8 subnormal avoidance)
ALPHA = 1.0 / 64.0  # ysum copy scale (fp16 overflow avoidance)
BETA = S * ALPHA  # denominator matmul constant = 0.5 (exact in fp8)
DR = mybir.MatmulPerfMode.DoubleRow

PHASE_MARKS = []


def _mark(nc, name):
    n = int(nc.get_next_instruction_name().split("-")[-1])
    PHASE_MARKS.append((n, name))


def build_nc(T, C):
    NQC = T // QC  # query chunks (4)
    NCC = C // P  # contraction chunks (16)
    KTQ = QC // P  # key tiles per query chunk (4)
    NKT = T // P  # key tiles total (16)
    exp_scale = 1.0 / (math.sqrt(HD) * S * S)

    nc = bacc.Bacc("TRN2", target_bir_lowering=False, debug=False,
                   num_devices=N_CORES)
    x8a = nc.dram_tensor("x8a", [C, T], F8, kind="ExternalInput").ap()
    x8b0 = nc.dram_tensor("x8b0", [C, QC], F8, kind="ExternalInput").ap()
    # pre-shuffled on host to [P, ...] so weight DMAs are contiguous copies
    NCC_ = C // P
    wq8 = nc.dram_tensor("wq8", [P, NCC_ * 2 * HPC * HD], F8,
                         kind="ExternalInput").ap()
    # wkv8 = [wk8a | wk8b | wv8a | wv8b], each [P, NCC*HD]
    wkv8 = nc.dram_tensor("wkv8", [P, 4 * NCC_ * HD], F8,
                          kind="ExternalInput").ap()
    y = nc.dram_tensor("y", [T, HPC * HD], F16, kind="ExternalOutput").ap()

    with tile.TileContext(nc) as tc, ExitStack() as ctx, \
            nc.allow_low_precision(reason="fp8 operands feed the PE; accumulation stays fp32 in PSUM"):
        consts = ctx.enter_context(tc.tile_pool(name="consts", bufs=1))
        ones8 = consts.tile([P, 2, 32], F8, tag="ones8")
        nc.gpsimd.memset(ones8, BETA)
        ones16 = consts.tile([P, 32], F16, tag="ones16")
        nc.gpsimd.memset(ones16, BETA)

        # ---- persistent SBUF ----
        big = ctx.enter_context(tc.tile_pool(name="big", bufs=1))
        x8a_sb = big.tile([P, NQC, NCC, QC], F8, tag="x8a")
        x8b0_sb = big.tile([P, NCC, QC], F8, tag="x8b0")
        wq8_sb = big.tile([P, NCC, 2 * HPC * HD], F8, tag="wq8")
        wkv8_sb = big.tile([P, 4, NCC, HD], F8, tag="wkv8")
        kT16 = big.tile([P, T], F16, tag="kT16")
        kT016 = big.tile([P, QC], F16, tag="kT016")
        v8 = big.tile([P, NKT, HD], F8, tag="v8")
        v016 = big.tile([P, KTQ, HD], F16, tag="v016")

        # ---- pools ----
        qT_pool = ctx.enter_context(tc.tile_pool(name="qT", bufs=3))
        pt_pool = ctx.enter_context(tc.tile_pool(name="pt", bufs=8))
        pt16_pool = ctx.enter_context(tc.tile_pool(name="pt16", bufs=3))
        vt_pool = ctx.enter_context(tc.tile_pool(name="vt", bufs=2))
        vtt_pool = ctx.enter_context(tc.tile_pool(name="vtt", bufs=2))
        ysum_pool = ctx.enter_context(tc.tile_pool(name="ysum", bufs=10))
        ysumt_pool = ctx.enter_context(tc.tile_pool(name="ysumt", bufs=5))
        sums_sb_pool = ctx.enter_context(tc.tile_pool(name="ssb", bufs=3))
        sumst_pool = ctx.enter_context(tc.tile_pool(name="sumst", bufs=3))
        yout_pool = ctx.enter_context(tc.tile_pool(name="yout", bufs=2))
        recip_pool = ctx.enter_context(tc.tile_pool(name="recip", bufs=3))

        # PSUM (8 banks): st 2x[128,2,512] = 4, y 2x[128,512] = 2,
        # sums 2x[32,512] = 2. Projections share st slots.
        st_pp = ctx.enter_context(tc.tile_pool(name="st_pp", bufs=2,
                                               space="PSUM"))
        y_pp = ctx.enter_context(tc.tile_pool(name="y_pp", bufs=2,
                                              space="PSUM"))
        sums_pp = ctx.enter_context(tc.tile_pool(name="sums_pp", bufs=2,
                                                 space="PSUM"))

        # ---- input DMAs: one queue, priority order, so the startup-critical
        # transfers (wkv8 + x chunk 0) are not bandwidth-starved ----
        xr = x8a.rearrange("(cc p) t -> p cc t", p=P)
        wkv_r = wkv8.rearrange("p (f cc d) -> p f cc d", f=4, cc=NCC)
        nc.sync.dma_start(out=wkv8_sb[:, 0], in_=wkv_r[:, 0])  # wk8a first
        nc.sync.dma_start(out=x8a_sb[:, 0, 0:NCC // 2], in_=xr[:, 0:NCC // 2, 0:QC])
        nc.sync.dma_start(out=x8a_sb[:, 0, NCC // 2:], in_=xr[:, NCC // 2:, 0:QC])
        nc.sync.dma_start(out=wkv8_sb[:, 1:], in_=wkv_r[:, 1:])
        nc.sync.dma_start(out=x8b0_sb,
                          in_=x8b0.rearrange("(cc p) t -> p cc t", p=P))
        nc.sync.dma_start(out=wq8_sb,
                          in_=wq8.rearrange("p (cc d) -> p cc d", cc=NCC))
        for tq in range(1, NQC):
            nc.sync.dma_start(out=x8a_sb[:, tq],
                              in_=xr[:, :, tq * QC:(tq + 1) * QC])

        def dr_proj(ps, w_sl, x_sl, first, last):
            # 8 DoubleRow matmuls: contraction C in pairs of 128-row tiles
            for c4 in range(NCC // 2):
                nc.tensor.matmul(
                    ps, lhsT=w_sl(c4), rhs=x_sl(c4),
                    start=(first and c4 == 0), stop=(last and c4 == NCC // 2 - 1),
                    perf_mode=DR)

        def wkv_slice(f):
            return lambda c4: wkv8_sb[:, f, 2 * c4:2 * c4 + 2]

        def wq_slice(term, h):
            off = term * HPC * HD + h * HD
            return lambda c4: wq8_sb[:, 2 * c4:2 * c4 + 2, off:off + HD]

        def x_slice(tq):
            return lambda c4: x8a_sb[:, tq, 2 * c4:2 * c4 + 2]

        def x0b_slice():
            return lambda c4: x8b0_sb[:, 2 * c4:2 * c4 + 2]

        def k_chunk(tq):
            ps = st_pp.tile([P, QC], F32, tag="st")
            dr_proj(ps, wkv_slice(0), x_slice(tq), True, True)
            nc.vector.tensor_copy(kT16[:, tq * QC:(tq + 1) * QC], ps)

        def v_chunk(tq, two_term):
            ps = st_pp.tile([P, QC], F32, tag="st")
            dr_proj(ps, wkv_slice(2), x_slice(tq), True, not two_term)
            if two_term:
                dr_proj(ps, wkv_slice(3), x_slice(tq), False, False)
                dr_proj(ps, wkv_slice(2), x0b_slice(), False, True)
            vt16 = vt_pool.tile([P, QC], F16, tag="vt")
            nc.vector.tensor_copy(vt16, ps)
            if two_term:
                nc.sync.dma_start_transpose(v016, vt16)
            else:
                vtt = vtt_pool.tile([P, KTQ, HD], F16, tag="vtt")
                nc.sync.dma_start_transpose(vtt, vt16)
                nc.vector.tensor_copy(v8[:, tq * KTQ:(tq + 1) * KTQ], vtt)

        with nc.named_scope("kv0"):
            _mark(nc, "kv0")
            ps0 = st_pp.tile([P, QC], F32, tag="st")
            dr_proj(ps0, wkv_slice(0), x_slice(0), True, False)
            dr_proj(ps0, wkv_slice(1), x_slice(0), False, False)
            dr_proj(ps0, wkv_slice(0), x0b_slice(), False, True)
            nc.vector.tensor_copy(kT016, ps0)
            v_chunk(0, True)

        # ---- Q projection per (chunk, head) ----
        qts = {}

        def emit_qproj(tq):
            _mark(nc, f"qproj{tq}")
            with nc.named_scope(f"qproj{tq}"):
                qTq = qT_pool.tile([P, HPC, QC], F16, tag="qT")
                for h in range(HPC):
                    ps = st_pp.tile([P, QC], F32, tag="st")
                    dr_proj(ps, wq_slice(0, h), x_slice(tq), True, tq != 0)
                    if tq == 0:  # three-term q for the low-key-count chunk
                        dr_proj(ps, wq_slice(1, h), x_slice(0), False, False)
                        dr_proj(ps, wq_slice(0, h), x0b_slice(), False, True)
                    nc.vector.tensor_copy(qTq[:, h], ps)
                qts[tq] = qTq

        pending_tails = []
        chunk_tail_state = {}

        def emit_tail(tq, hp, ysum2, sums2):
            # Per head-pair, deferred one chunk. PE-free: transposes via the
            # DMA XBAR. One sums transpose + one output DMA per chunk.
            with nc.named_scope(f"ltail{tq}p{hp}"):
                _mark(nc, f"q{tq}:ltail{hp}")
                if tq not in chunk_tail_state:
                    yo = yout_pool.tile([P, KTQ, HPC, HD], F16, tag="yo")
                    if tq < NQC - 1:
                        # deferred chunk: both pairs' denominator rows are
                        # already written -> one transpose + 4 reciprocals
                        sumst = sumst_pool.tile([P, KTQ, P], F16, tag="sumst")
                        nc.sync.dma_start_transpose(sumst, sums2)
                        rtc = recip_pool.tile([P, KTQ, HPC], F32, tag="recip")
                        for th in range(HPC):
                            nc.vector.reciprocal(
                                rtc[:, :, th:th + 1],
                                sumst[:, :, th * 32:th * 32 + 1])
                    else:
                        rtc = None
                    chunk_tail_state[tq] = (yo, rtc)
                yo, rtc = chunk_tail_state[tq]
                if rtc is None:
                    # last chunk drains eagerly per pair: transpose this
                    # pair's rows only
                    sumst = sumst_pool.tile([P, KTQ, 64], F16, tag="sumst")
                    nc.sync.dma_start_transpose(
                        sumst, sums2[hp * 64:hp * 64 + 64, :])
                    rt = recip_pool.tile([P, KTQ, 2], F32, tag="recip2")
                    for i in range(2):
                        nc.vector.reciprocal(rt[:, :, i:i + 1],
                                             sumst[:, :, i * 32:i * 32 + 1])
                else:
                    rt = rtc[:, :, 2 * hp:2 * hp + 2]
                yst = ysumt_pool.tile([P, 2 * KTQ, HD], F16, tag="yst")
                nc.scalar.dma_start_transpose(
                    yst, ysum2.rearrange("p two q -> p (two q)"))
                for i in range(2):
                    for qt in range(KTQ):
                        nc.vector.tensor_scalar_mul(
                            yo[:, qt, 2 * hp + i], yst[:, i * KTQ + qt],
                            rt[:, qt, i:i + 1])
                if hp == 1:
                    ydst = y.rearrange("(nq qt p) (hh d) -> nq p qt hh d",
                                       qt=KTQ, p=P, hh=HPC)[tq]
                    nc.sync.dma_start(out=ydst, in_=yo)

        def emit_attn(tq):
            # ---- causal attention for this query chunk ----
            # Head PAIRS: the PE alternates score and PV matmuls between the
            # two heads so each head's exp chain hides under the other's
            # matmuls. Chunk 0 uses the accurate fp16 path.
            last_chunk = tq == NQC - 1
            fp16_path = tq == 0
            nkt = (tq + 1) * KTQ
            ngr = nkt // 2  # key-tile pairs
            qTq = qts.pop(tq)
            sums2 = sums_sb_pool.tile([P, QC], F16, tag="ssb")
            kt_src = kT016 if fp16_path else kT16

            def s_mm(h, g):
                # scores + exp (+ diag mask) for key-tile pair g
                st = st_pp.tile([P, 2, QC], F32, tag="st")
                if fp16_path:
                    pt = pt16_pool.tile([P, 2, QC], F16, tag="pt16")
                else:
                    pt = pt_pool.tile([P, 2, QC], F8, tag="pt")
                qrhs = qTq[:, h]
                offs = []
                for u in range(2):
                    kt = 2 * g + u
                    off = max((kt - KTQ * tq) * P, 0)
                    offs.append(off)
                    nc.tensor.matmul(
                        st[:, u, off:],
                        lhsT=kt_src[:, kt * P:(kt + 1) * P],
                        rhs=qrhs[:, off:], start=True, stop=True)
                o0, o1 = offs
                nc.scalar.activation(pt[:, :, o0:], st[:, :, o0:],
                                     mybir.ActivationFunctionType.Exp,
                                     scale=exp_scale)
                if 2 * g + 1 >= KTQ * tq:  # pair contains diagonal tiles
                    if o1 > o0:
                        nc.gpsimd.memset(pt[:, 1, o0:o1], 0.0)
                    for u, off in enumerate(offs):
                        nc.gpsimd.affine_select(
                            out=pt[:, u, off:off + P],
                            in_=pt[:, u, off:off + P],
                            pattern=[[1, P]],
                            compare_op=mybir.AluOpType.is_ge,
                            fill=0.0, base=0, channel_multiplier=-1)
                return pt, o0

            for hp in range(HPC // 2):
              pair = (2 * hp, 2 * hp + 1)
              with nc.named_scope(f"attn{tq}p{hp}"):
                y_psh = {}
                s_psh = {}
                pts = {}
                for h in pair:
                    y_ps = y_pp.tile([P, QC], F32, tag="y")
                    y_psh[h] = y_ps
                    s_ps = sums_pp.tile([32, QC], F32, tag="sums")
                    s_psh[h] = s_ps
                    pts[h] = {0: s_mm(h, 0)}
                for g in range(ngr):
                    _mark(nc, f"q{tq}:att{hp}g{g}")
                    if g + 1 < ngr:
                        for h in pair:
                            pts[h][g + 1] = s_mm(h, g + 1)
                    for h in pair:
                        pt, o0 = pts[h].pop(g)
                        first, last = g == 0, g == ngr - 1
                        if fp16_path:
                            for u in range(2):
                                kt = 2 * g + u
                                off = max((kt - KTQ * tq) * P, 0)
                                nc.tensor.matmul(
                                    y_psh[h][:, off:], lhsT=v016[:, kt],
                                    rhs=pt[:, u, off:],
                                    start=(kt == 0), stop=(kt == nkt - 1),
                                    skip_group_check=True)
                                nc.tensor.matmul(
                                    s_psh[h][:, off:], lhsT=ones16,
                                    rhs=pt[:, u, off:],
                                    start=(kt == 0), stop=(kt == nkt - 1),
                                    skip_group_check=True)
                        else:
                            v_sl = v8.rearrange("p (gg two) d -> p gg two d",
                                                two=2)[:, g]
                            nc.tensor.matmul(
                                y_psh[h][:, o0:], lhsT=v_sl,
                                rhs=pt[:, :, o0:],
                                start=first, stop=last, perf_mode=DR,
                                skip_group_check=True)
                            nc.tensor.matmul(
                                s_psh[h][:, o0:], lhsT=ones8,
                                rhs=pt[:, :, o0:],
                                start=first, stop=last, perf_mode=DR,
                                skip_group_check=True)
                _mark(nc, f"q{tq}:tail{hp}")
                ysum2 = ysum_pool.tile([P, 2, QC], F16, tag="ysum")
                for i, h in enumerate(pair):
                    nc.vector.tensor_scalar_mul(ysum2[:, i], y_psh[h], ALPHA)
                    nc.vector.tensor_copy(sums2[h * 32:h * 32 + 1, :],
                                          s_psh[h][0:1, :])
                pending_tails.append((tq, hp, ysum2, sums2))
                # keep <= 2 pending (pipelines tails under the next chunk's
                # matmuls); on the last chunk drain eagerly so each tail
                # overlaps the remaining pairs' matmuls
                limit = 0 if last_chunk else 2
                while len(pending_tails) > limit:
                    emit_tail(*pending_tails.pop(0))

        # ---- emission order: minimal work before attn0 (K0s/V0s are only
        # needed from attn1 on, so they ride with kv1) ----
        emit_qproj(0)
        emit_attn(0)
        for tq in range(1, NQC):
            with nc.named_scope(f"kv{tq}"):
                _mark(nc, f"kv{tq}")
                if tq == 1:
                    k_chunk(0)
                    v_chunk(0, False)
                k_chunk(tq)
                v_chunk(tq, False)
            emit_qproj(tq)
            emit_attn(tq)

    nc.compile()
    return nc


_cache = {}


def _get_nc(T, C):
    key = (T, C)
    if key not in _cache:
        _cache[key] = build_nc(T, C)
    return _cache[key]


def prepare_in_maps(x, w_kv, w_q):
    x = np.asarray(x, dtype=np.float32)
    w_kv = np.asarray(w_kv, dtype=np.float32)
    w_q = np.asarray(w_q, dtype=np.float32)
    B, T, C = x.shape

    NCC = C // P

    def two_term(w):  # [C, D] scaled two-term fp8
        ws = np.ascontiguousarray(w) * S
        a = ws.astype(E4M3)
        b = (ws - a.astype(np.float32)).astype(E4M3)
        return a, b

    def shuffle(w8):  # [C, D] -> [P, NCC*D] matching [p, cc, d] tiles
        D = w8.shape[1]
        return w8.reshape(NCC, P, D).transpose(1, 0, 2).reshape(P, NCC * D)

    wk = w_kv[:HD].T  # [C, HD]
    wv = w_kv[HD:].T
    wk8a, wk8b = two_term(wk)
    wv8a, wv8b = two_term(wv)
    wkv8 = np.ascontiguousarray(np.concatenate(
        [shuffle(w) for w in (wk8a, wk8b, wv8a, wv8b)], axis=1))

    in_maps = []
    for i in range(N_CORES):
        b, hg = divmod(i, NB)
        xT = np.ascontiguousarray(x[b].T)  # [C, T]
        x8a = xT.astype(E4M3)
        x8b0 = (xT[:, :QC] - x8a[:, :QC].astype(np.float32)).astype(E4M3)
        wqs = w_q[hg * HPC * HD:(hg + 1) * HPC * HD].T  # [C, 512]
        wq8a, wq8b = two_term(wqs)
        # interleave per-cc: [P, NCC, 2*512] flattened
        sa = shuffle(wq8a).reshape(P, NCC, HPC * HD)
        sb = shuffle(wq8b).reshape(P, NCC, HPC * HD)
        wq8 = np.ascontiguousarray(
            np.concatenate([sa, sb], axis=2).reshape(P, NCC * 2 * HPC * HD))
        in_maps.append({"x8a": x8a, "x8b0": np.ascontiguousarray(x8b0),
                        "wq8": wq8, "wkv8": wkv8})
    return in_maps


def gather_output(results, B, T, C):
    out = np.empty((B, T, C), np.float32)
    for i in range(N_CORES):
        b, hg = divmod(i, NB)
        out[b, :, hg * HPC * HD:(hg + 1) * HPC * HD] = \
            results[i]["y"].astype(np.float32)
    return out


def kernel(x, w_kv, w_q):
    x = np.asarray(x)
    B, T, C = x.shape
    nc = _get_nc(T, C)
    in_maps = prepare_in_maps(x, w_kv, w_q)
    res = run_bass_kernel_spmd(nc, in_maps, list(range(N_CORES)))
    return gather_output(res.results, B, T, C)
